# revision 56
# baseline (speedup 1.0000x reference)
"""BiBoMoE layer (15 SwiGLU experts + identity expert + shared conv expert, top-2 of 16)
on 8 TRN2 NeuronCores.

Strategy: data-parallel over tokens (each core owns 2048 of the 16384 tokens, expert
weights replicated). Two device passes:
  pass 1: fp32 router matmul + softmax/top-2 on device; raw top-2 (ids + normalized
          weights) shipped to host, which builds all dispatch lists (pure logistics).
  pass 2 (compiled with the exact per-expert counts from pass 1): one fused program:
          - shared causal-conv expert in fp16, dense over all tokens (too
            error-amplifying for fp8: its output is ~half the final magnitude);
          - 15 SwiGLU experts entirely in fp8(e4m3) DoubleRow matmuls (host
            pre-quantized weights + activations; per-channel gate/up weight
            scales undone at PSUM eviction: gate inside silu's per-partition
            scale operand, up inside the h=silu(g)*u product, down's per-tensor
            scale folded into the per-token gating weight at eviction);
          - identity expert via host-pregathered rows, scale only.
          Every contribution is written with dma_scatter_add into the single
          zero-initialized fp32 output (shared tiles use identity indices).
          Expert scatters form one explicit dependency chain (shared-tile
          scatters are row-disjoint and only order against expert scatters), so
          the read-modify-write scatters never race; per-core count shortfalls
          are padded with gating-0 entries aimed at a trash row so they cannot
          collide with live rows either. DMA is split across both HWDGE queues
          (SP: weights + shared consts; ACT: activation slices with a rolling
          prefetch), shared tiles interleaved between expert items to balance
          the PE-heavy conv against the DMA-heavy experts.
No collectives: cores never communicate; host splits tokens and concatenates outputs.
"""
import sys

sys.path.insert(0, "/opt/trn_rl_repo")

import numpy as np
import ml_dtypes

import concourse.bass as bass
import concourse.bacc as bacc
import concourse.tile as tile
from concourse import mybir
from concourse.bass_utils import run_bass_kernel_spmd
from concourse.tile import add_dep_helper

FP32 = mybir.dt.float32
FP16 = mybir.dt.float16
FP8 = mybir.dt.float8e4
I16 = mybir.dt.int16
U32 = mybir.dt.uint32
AF = mybir.ActivationFunctionType
AX = mybir.AxisListType
ALU = mybir.AluOpType
PM = mybir.MatmulPerfMode
NPF8 = ml_dtypes.float8_e4m3  # trn float8e4: e4m3, max 240

B, S, H, I, E, TOPK, KS = 4, 4096, 1024, 512, 16, 2, 3
NCORES = 8
T = B * S            # 16384 tokens
TC = T // NCORES     # 2048 tokens per core
NBI = TC // 128      # 16 token groups per core (pass 1)
HJ = H // 128        # 8 H-chunks
MI = I // 128        # 4 I-chunks
NEXP = E - 1         # 15 MLP experts; expert 15 is identity
F8MAX = 192.0        # quantization target amax (margin under e4m3 max 240)


def _wrap_idxs(idx_list, cap):
    """Scatter-index layout [128, cap//16] int16 (position i -> [i%16 row-rep x8, i//16])."""
    a = np.zeros(cap, dtype=np.int16)
    a[: len(idx_list)] = idx_list
    return np.tile(a.reshape(-1, 16).T, (8, 1)).copy()


def _gate_cols(g_list, cap):
    """[128, cap//128] fp32: position i=(j*128+p) -> [p, j]."""
    a = np.zeros(cap, dtype=np.float32)
    a[: len(g_list)] = g_list
    return np.ascontiguousarray(a.reshape(-1, 128).T)


def _build_pass1():
    """Router: logits + softmax + top-2 normalized weights; raw results to host."""
    nc = bacc.Bacc("TRN2", target_bir_lowering=False, debug=False, num_devices=NCORES)
    xT_d = nc.dram_tensor("xT", [H, TC], FP32, kind="ExternalInput")
    rw_d = nc.dram_tensor("rw", [H, E], FP32, kind="ExternalInput")
    rb_d = nc.dram_tensor("rb", [1, E], FP32, kind="ExternalInput")
    top_o = nc.dram_tensor("top", [128, NBI * 2], FP32, kind="ExternalOutput")
    arg_o = nc.dram_tensor("arg", [128, NBI * 2], U32, kind="ExternalOutput")

    with tile.TileContext(nc) as tc:
        with (
            tc.tile_pool(name="big", bufs=1) as big,
            tc.tile_pool(name="small", bufs=2) as small,
            tc.tile_pool(name="psum", bufs=2, space=bass.MemorySpace.PSUM) as psum,
        ):
            rw_t = big.tile([128, HJ, E], FP32)
            nc.sync.dma_start(rw_t[:], rw_d.ap().rearrange("(c p) e -> p c e", p=128))
            rb1_t = big.tile([1, E], FP32)
            nc.sync.dma_start(rb1_t[:], rb_d[:])
            rb_t = big.tile([128, E], FP32)
            nc.gpsimd.partition_broadcast(rb_t[:], rb1_t[:])
            # x loaded per-group so the first matmul starts early
            xT_t = big.tile([128, HJ, TC], FP32)
            xT_src = xT_d.ap().rearrange("(c p) t -> p c t", p=128)
            for gi, g in enumerate(range(0, NBI, 2)):
                eng = nc.sync if gi % 2 == 0 else nc.scalar
                eng.dma_start(
                    xT_t[:, :, g * 128 : (g + 2) * 128],
                    xT_src[:, :, g * 128 : (g + 2) * 128],
                )

            top_t = big.tile([128, NBI, 2], FP32)
            arg_t = big.tile([128, NBI, 2], U32)
            xT_r = xT_t[:].rearrange("p c (b q) -> p c b q", b=NBI)

            for bi in range(NBI):
                # token t = bi*128 + q sits on psum partition q
                lp = psum.tile([128, E], FP32)
                for hj in range(HJ):
                    nc.tensor.matmul(
                        lp[:],
                        xT_r[:, hj, bi, :],
                        rw_t[:, hj, :],
                        start=(hj == 0),
                        stop=(hj == HJ - 1),
                    )
                l_t = small.tile([128, E], FP32)
                nc.vector.tensor_tensor(l_t[:], lp[:], rb_t[:], op=ALU.add)
                lv = small.tile([128, 8], FP32)
                li = small.tile([128, 8], U32)
                nc.vector.max_with_indices(lv[:], li[:], l_t[:])
                nm = small.tile([128, 1], FP32)
                nc.vector.tensor_scalar_mul(nm[:], lv[:, 0:1], -1.0)
                e_t = small.tile([128, E], FP32)
                z_t = small.tile([128, 1], FP32)
                nc.scalar.activation(e_t[:], l_t[:], AF.Exp, bias=nm[:], accum_out=z_t[:])
                e2 = small.tile([128, 2], FP32)
                nc.scalar.activation(e2[:], lv[:, 0:2], AF.Exp, bias=nm[:])
                s2 = small.tile([128, 1], FP32)
                nc.vector.tensor_reduce(s2[:], e2[:], axis=AX.X, op=ALU.add)
                d_t = small.tile([128, 1], FP32)
                nc.vector.scalar_tensor_tensor(
                    d_t[:], z_t[:], 1e-6, s2[:], op0=ALU.mult, op1=ALU.add
                )
                r_t = small.tile([128, 1], FP32)
                nc.vector.reciprocal(r_t[:], d_t[:])
                nc.vector.tensor_scalar_mul(top_t[:, bi, :], e2[:], r_t[:])
                nc.vector.tensor_copy(arg_t[:, bi, :], li[:, 0:2])

            nc.sync.dma_start(top_o[:], top_t[:].rearrange("p b k -> p (b k)"))
            nc.sync.dma_start(arg_o[:], arg_t[:].rearrange("p b k -> p (b k)"))
    nc.compile()
    return nc


def _build_pass2(work, fp8_down=True):
    """work: list of (expert_id, cap, sz) items. Experts with more tokens than 512
    are pre-split into <=512 chunks. cap = padded row capacity (multiple of 128),
    sz = scattered row count (max over cores; per-core shortfall padded with
    gating-0 entries)."""
    nc = bacc.Bacc("TRN2", target_bir_lowering=False, debug=False, num_devices=NCORES,
                   num_swdge_queues=4)
    xTh_d = nc.dram_tensor("xTh", [H, TC + 2], FP16, kind="ExternalInput")
    convw_d = nc.dram_tensor("convw", [H, KS, I], FP16, kind="ExternalInput")
    swu_d = nc.dram_tensor("swu", [H, I], FP16, kind="ExternalInput")
    swd_d = nc.dram_tensor("swd", [I, H], FP16, kind="ExternalInput")
    wgu_d = nc.dram_tensor("wgu", [NEXP, 2 * H, I], FP8, kind="ExternalInput")
    wd_d = nc.dram_tensor("wd", [NEXP, I, H], FP8 if fp8_down else FP16,
                          kind="ExternalInput")
    caps = [c for (_, c, _) in work]
    xcap = sum(caps)
    idcap = max(sum(c for (e, c, _) in work if e == E - 1), 128)
    idoff = {}
    _o = 0
    for w, (e, c, _) in enumerate(work):
        if e == E - 1:
            idoff[w] = _o
            _o += c
    x8_d = nc.dram_tensor("x8", [128, HJ * xcap], FP8, kind="ExternalInput")
    xid_d = nc.dram_tensor("xid", [128, idcap // 128, H], FP16, kind="ExternalInput")
    # merged index table (idx | ids) and fp32 table (gcol | gsc | usc)
    tabi_d = nc.dram_tensor("tabi", [128, xcap // 16 + TC // 16], I16,
                            kind="ExternalInput")
    tabf_d = nc.dram_tensor("tabf", [128, xcap // 128 + 2 * NEXP * MI], FP32,
                            kind="ExternalInput")
    out_d = nc.dram_tensor("out", [TC + 1, H], FP32, kind="ExternalOutput")

    off16 = [sum(caps[:w]) // 16 for w in range(len(work))]
    off128 = [sum(caps[:w]) // 128 for w in range(len(work))]
    offx = [sum(caps[:w]) for w in range(len(work))]

    HDT = FP8 if fp8_down else FP16
    TT = 512  # shared-expert token tile
    NSH = TC // TT
    # schedule: spread the PE-heavy shared tiles between the DMA-heavy expert
    # items so both engines stay busy throughout. Identity item (no matmuls)
    # first as warm-up while weights stream in; largest experts next so the
    # chain tail is a small item.
    expd = sorted([w for w in range(len(work)) if work[w][0] != E - 1],
                  key=lambda w: -work[w][2])
    idw = [w for w in range(len(work)) if work[w][0] == E - 1]
    ids_sorted = idw + expd
    prog = []
    tt_next = 0
    for i, w in enumerate(ids_sorted):
        prog.append(("exp", w))
        if i in (2, 6, 10, 15) and tt_next < NSH:
            prog.append(("sh", tt_next))
            tt_next += 1
    while tt_next < NSH:
        prog.append(("sh", tt_next))
        tt_next += 1

    with tile.TileContext(nc) as tc:
        with (
            tc.tile_pool(name="const", bufs=1) as const,
            tc.tile_pool(name="xs", bufs=2) as xs,
            tc.tile_pool(name="hb", bufs=2) as hb,
            tc.tile_pool(name="wexp", bufs=3) as wexp,
            tc.tile_pool(name="xg", bufs=4) as xgp,
            tc.tile_pool(name="sc", bufs=3) as scp,
            tc.tile_pool(name="so", bufs=1) as sop,
            tc.tile_pool(name="ps", bufs=2, space=bass.MemorySpace.PSUM) as ps,
            tc.tile_pool(name="psc", bufs=1, space=bass.MemorySpace.PSUM) as psc,
            tc.tile_pool(name="psd", bufs=2, space=bass.MemorySpace.PSUM) as psd,
        ):
            tabi_t = const.tile([128, xcap // 16 + TC // 16], I16)
            tabf_t = const.tile([128, xcap // 128 + 2 * NEXP * MI], FP32)
            idx_t = tabi_t[:, 0 : xcap // 16]
            ids_t = tabi_t[:, xcap // 16 :]
            gcol_t = tabf_t[:, 0 : xcap // 128]
            gsc_t = tabf_t[:, xcap // 128 : xcap // 128 + NEXP * MI]
            usc_t = tabf_t[:, xcap // 128 + NEXP * MI :]
            xid_t = const.tile([128, idcap // 128, H], FP16)
            convw_t = const.tile([128, HJ, KS, I], FP16)
            swu_t = const.tile([128, HJ, I], FP16)
            swd_t = const.tile([128, MI, H], FP16)
            xid_loaded = [False]

            def load_xid():
                if not xid_loaded[0]:
                    xid_loaded[0] = True
                    nc.scalar.dma_start(xid_t[:], xid_d[:])
            consts_step = [0]

            def load_shared_consts():
                # chunked: each call ships ~1MB so no queue camps for long
                s = consts_step[0]
                if s > 4:
                    return
                consts_step[0] += 1
                if s < 3:
                    nc.sync.dma_start(
                        convw_t[:, :, s, :],
                        convw_d.ap().rearrange("(c p) k i -> p c k i", p=128)[:, :, s, :],
                    )
                elif s == 3:
                    nc.sync.dma_start(
                        swu_t[:], swu_d.ap().rearrange("(c p) i -> p c i", p=128))
                else:
                    nc.sync.dma_start(
                        swd_t[:], swd_d.ap().rearrange("(c p) h -> p c h", p=128))

            xTh_src = xTh_d.ap().rearrange("(c p) t -> p c t", p=128)
            xw_tiles = {}
            exp_order = [a for (k, a) in prog if k == "exp"]
            xg_tiles = {}

            def prefetch_xg(j):
                while j < len(exp_order):
                    w = exp_order[j]
                    if work[w][0] != E - 1:
                        break
                    j += 1
                else:
                    return
                if w in xg_tiles:
                    return
                e_, cap_, _ = work[w]
                xgt = xgp.tile([128, HJ, cap_], FP8, tag="xg")
                nc.scalar.dma_start(
                    xgt[:],
                    x8_d.ap()[:, HJ * offx[w] : HJ * (offx[w] + cap_)]
                    .rearrange("p (c t) -> p c t", c=HJ),
                )
                xg_tiles[w] = xgt

            prefetch_xg(0)
            nc.scalar.dma_start(tabi_t[:], tabi_d[:])
            nc.scalar.dma_start(tabf_t[:], tabf_d[:])
            load_xid()
            for j0 in range(1, 4):
                prefetch_xg(j0)
            exp_pos = {w: j for j, w in enumerate(exp_order)}

            def prefetch_xw(tt):
                if tt in xw_tiles or tt >= NSH:
                    return
                xw = xs.tile([128, HJ, TT + 2], FP16, tag="xw")
                nc.sync.dma_start(
                    xw[:], xTh_src[:, :, tt * TT : tt * TT + TT + 2])
                xw_tiles[tt] = xw

            # Out-writer ordering: expert/identity scatters form a chain; the
            # shared tiles' identity-indexed scatters are mutually row-disjoint,
            # so they only order against expert scatters (both directions), not
            # against each other.
            last_sc = []   # last expert scatter
            pend_so = []   # so-scatters since the last expert scatter
            qrr = [0]

            def chained_scatter(in_ap, idxs_ap, n, disjoint=False):
                qrr[0] = (qrr[0] + 1) % 4
                i_s = nc.gpsimd.dma_scatter_add(
                    out_ap=out_d[:],
                    in_ap=in_ap,
                    idxs_ap=idxs_ap,
                    num_idxs=n,
                    num_idxs_reg=n,
                    elem_size=H,
                    queue_num=qrr[0],
                )
                if disjoint:
                    if last_sc:
                        add_dep_helper(i_s.ins, last_sc[0].ins, reason="so after sc")
                    pend_so.append(i_s)
                else:
                    if last_sc:
                        add_dep_helper(i_s.ins, last_sc[0].ins, reason="sc chain")
                    for p in pend_so:
                        add_dep_helper(i_s.ins, p.ins, reason="sc after so group")
                    pend_so.clear()
                    last_sc[:] = [i_s]

            for pi, (kind, arg) in enumerate(prog):
                for kind2, arg2 in prog[pi + 1 : pi + 3]:
                    if kind2 == "sh":
                        prefetch_xw(arg2)
                if kind == "sh":
                    tt = arg
                    xw = xw_tiles.pop(tt)
                    hs = hb.tile([128, MI, TT], FP16, tag="hs")
                    for mi in range(MI):
                        pg = psc.tile([128, TT], FP32, tag="cpg")
                        for k in range(KS):
                            for hj in range(HJ):
                                nc.tensor.matmul(
                                    pg[:],
                                    convw_t[:, hj, k, mi * 128 : mi * 128 + 128],
                                    xw[:, hj, k : k + TT],
                                    start=(k == 0 and hj == 0),
                                    stop=(k == KS - 1 and hj == HJ - 1),
                                )
                        pu = psc.tile([128, TT], FP32, tag="cpu")
                        for hj in range(HJ):
                            nc.tensor.matmul(
                                pu[:],
                                swu_t[:, hj, mi * 128 : mi * 128 + 128],
                                xw[:, hj, 2 : 2 + TT],
                                start=(hj == 0),
                                stop=(hj == HJ - 1),
                            )
                        sg = hb.tile([128, TT], FP16, tag="sg")
                        nc.scalar.activation(sg[:], pg[:], AF.Silu)
                        nc.vector.tensor_tensor(hs[:, mi, :], sg[:], pu[:], op=ALU.mult)
                    so = sop.tile([128, TT // 128, H], FP32, tag="so")
                    for tb in range(TT // 128):
                        for hh in range(2):
                            py = psd.tile([128, 512], FP32, tag="py")
                            for mi in range(MI):
                                nc.tensor.matmul(
                                    py[:],
                                    hs[:, mi, tb * 128 : tb * 128 + 128],
                                    swd_t[:, mi, hh * 512 : hh * 512 + 512],
                                    start=(mi == 0),
                                    stop=(mi == MI - 1),
                                )
                            nc.vector.tensor_copy(so[:, tb, hh * 512 : hh * 512 + 512], py[:])
                    for hf in range(2):
                        chained_scatter(
                            so[:, hf * 2 : hf * 2 + 2, :],
                            ids_t[:, tt * (TT // 16) + hf * 16 : tt * (TT // 16) + hf * 16 + 16],
                            TT // 2,
                            disjoint=True)
                    continue

                wi = arg
                e, cap, sz = work[wi]
                if sz == 0:
                    continue
                if e == E - 1:
                    # identity expert: host-pregathered rows, scale, scatter
                    j0 = idoff[wi] // 128
                    sci = scp.tile([128, cap // 128, H], FP32, tag="sc")
                    for j in range(-(-sz // 128)):
                        nc.vector.tensor_scalar_mul(
                            sci[:, j, :],
                            xid_t[:, j0 + j, :],
                            gcol_t[:, off128[wi] + j : off128[wi] + j + 1],
                        )
                    chained_scatter(
                        sci[:, 0 : -(-sz // 128), :],
                        idx_t[:, off16[wi] : off16[wi] + -(-sz // 16)],
                        sz,
                    )
                    continue
                wgu_t = wexp.tile([128, 2 * HJ, I], FP8, tag="wgu")
                nc.sync.dma_start(
                    wgu_t[:], wgu_d.ap()[e].rearrange("(c p) i -> p c i", p=128)
                )

                wd_t = wexp.tile([128, MI, H], HDT, tag="wd")
                nc.sync.dma_start(
                    wd_t[:], wd_d.ap()[e].rearrange("(c p) h -> p c h", p=128)
                )
                if wi not in xg_tiles:
                    prefetch_xg(exp_pos[wi])
                xg = xg_tiles.pop(wi)

                n = sz  # live rows only; tail rows beyond sz are never scattered
                hx = hb.tile([128, MI, cap], HDT, tag="hx")
                for mi in range(MI):
                    pg = ps.tile([128, 512], FP32, tag="pg")
                    for hj in range(0, HJ, 2):
                        nc.tensor.matmul(
                            pg[:, 0:n],
                            wgu_t[:, hj : hj + 2, mi * 128 : mi * 128 + 128],
                            xg[:, hj : hj + 2, 0:n],
                            start=(hj == 0),
                            stop=(hj == HJ - 2),
                            perf_mode=PM.DoubleRow,
                        )
                    pu = ps.tile([128, 512], FP32, tag="pu")
                    for hj in range(0, HJ, 2):
                        nc.tensor.matmul(
                            pu[:, 0:n],
                            wgu_t[:, HJ + hj : HJ + hj + 2, mi * 128 : mi * 128 + 128],
                            xg[:, hj : hj + 2, 0:n],
                            start=(hj == 0),
                            stop=(hj == HJ - 2),
                            perf_mode=PM.DoubleRow,
                        )
                    sg = hb.tile([128, 512], FP16, tag="sgx")
                    nc.scalar.activation(
                        sg[:, 0:n], pg[:, 0:n], AF.Silu,
                        scale=gsc_t[:, e * MI + mi : e * MI + mi + 1],
                    )
                    nc.vector.scalar_tensor_tensor(
                        hx[:, mi, 0:n],
                        pu[:, 0:n],
                        usc_t[:, e * MI + mi : e * MI + mi + 1],
                        sg[:, 0:n],
                        op0=ALU.mult,
                        op1=ALU.mult,
                    )
                sc = scp.tile([128, cap // 128, H], FP32, tag="sc")
                for tb in range(-(-sz // 128)):
                    tn = min(128, sz - tb * 128)
                    for hh in range(2):
                        py = psd.tile([128, 512], FP32, tag="py")
                        if fp8_down:
                            for mi in range(0, MI, 2):
                                nc.tensor.matmul(
                                    py[0:tn, :],
                                    hx[:, mi : mi + 2, tb * 128 : tb * 128 + tn],
                                    wd_t[:, mi : mi + 2, hh * 512 : hh * 512 + 512],
                                    start=(mi == 0),
                                    stop=(mi == MI - 2),
                                    perf_mode=PM.DoubleRow,
                                )
                        else:
                            for mi in range(MI):
                                nc.tensor.matmul(
                                    py[0:tn, :],
                                    hx[:, mi, tb * 128 : tb * 128 + tn],
                                    wd_t[:, mi, hh * 512 : hh * 512 + 512],
                                    start=(mi == 0),
                                    stop=(mi == MI - 1),
                                )
                        nc.vector.tensor_scalar_mul(
                            sc[0:tn, tb, hh * 512 : hh * 512 + 512],
                            py[0:tn, :],
                            gcol_t[0:tn, off128[wi] + tb : off128[wi] + tb + 1],
                        )
                chained_scatter(
                    sc[:, 0 : -(-sz // 128), :],
                    idx_t[:, off16[wi] : off16[wi] + -(-sz // 16)],
                    sz,
                )
                prefetch_xg(exp_pos[wi] + 3)
                load_shared_consts()
                if pi == 1:
                    load_shared_consts()
                elif pi == 2:
                    load_shared_consts()
                    load_shared_consts()

    nc.compile()
    return nc


def kernel(
    hidden_states,
    router_w,
    router_bias,
    expert_gate_w,
    expert_up_w,
    expert_down_w,
    conv_w,
    shared_up_w,
    shared_down_w,
    fp8_down=True,
    dry=False,
):
    hidden_states = np.asarray(hidden_states, dtype=np.float32)
    flat = np.ascontiguousarray(hidden_states.reshape(T, H))
    cores = list(range(NCORES))

    # ---------------- pass 1: router ----------------------------------------------
    nc1 = _build_pass1()
    rw32 = np.asarray(router_w, dtype=np.float32)
    rb32 = np.asarray(router_bias, dtype=np.float32).reshape(1, E)
    in_maps1 = []
    for c in cores:
        xs = flat[c * TC : (c + 1) * TC]
        in_maps1.append({"xT": np.ascontiguousarray(xs.T), "rw": rw32, "rb": rb32})
    global NC1, IN_MAPS1
    NC1, IN_MAPS1 = nc1, in_maps1
    if dry:
        logits = flat @ rw32 + rb32
        pr = np.exp(logits - logits.max(-1, keepdims=True))
        pr /= pr.sum(-1, keepdims=True)
        tidx = np.argsort(-pr, -1)[:, :TOPK]
        tval = np.take_along_axis(pr, tidx, -1)
        nwt = (tval / (tval.sum(-1, keepdims=True) + 1e-6)).astype(np.float32)
        res1 = None
    else:
        res1 = run_bass_kernel_spmd(nc1, in_maps1, cores).results

    # ---------------- host: decode top-2, build per-expert lists -------------------
    per_core_lists = []
    for c in cores:
        if dry:
            ti = tidx[c * TC : (c + 1) * TC].astype(np.int64)
            tw = nwt[c * TC : (c + 1) * TC]
        else:
            top = res1[c]["top"].reshape(128, NBI, 2)
            arg = res1[c]["arg"].reshape(128, NBI, 2)
            # token t = bi*128 + q -> [q, bi]
            ti = np.transpose(arg, (1, 0, 2)).reshape(TC, 2).astype(np.int64)
            tw = np.transpose(top, (1, 0, 2)).reshape(TC, 2).astype(np.float32)
        lists = []
        for e in range(E):
            m0 = ti[:, 0] == e
            m1 = ti[:, 1] == e
            toks = np.concatenate([np.nonzero(m0)[0], np.nonzero(m1)[0]])
            gats = np.concatenate([tw[m0, 0], tw[m1, 1]]).astype(np.float32)
            o = np.argsort(toks, kind="stable")
            lists.append((toks[o], gats[o]))
        per_core_lists.append(lists)

    maxcnt = [max(len(per_core_lists[c][e][0]) for c in cores) for e in range(E)]
    work = []  # (expert, cap, sz) ; chunk k covers list rows [k*512, k*512+sz)
    chunk0 = []
    for e in range(E):
        nch = max(1, -(-maxcnt[e] // 512))
        for k in range(nch):
            szk = max(0, min(512, maxcnt[e] - k * 512))
            cap = max(128, -(-szk // 128) * 128)
            work.append((e, cap, szk))
            chunk0.append(k * 512)

    # ---------------- host: quantization + all pass-2 arrays -----------------------
    wg32 = np.asarray(expert_gate_w, dtype=np.float32)
    wu32 = np.asarray(expert_up_w, dtype=np.float32)
    sx = F8MAX / max(float(np.abs(flat).max()), 1e-9)
    sgw = F8MAX / np.maximum(np.abs(wg32).max(axis=1), 1e-9)   # (NEXP, I)
    suw = F8MAX / np.maximum(np.abs(wu32).max(axis=1), 1e-9)   # (NEXP, I)
    wg8 = np.clip(wg32 * sgw[:, None, :], -240, 240).astype(NPF8)
    wu8 = np.clip(wu32 * suw[:, None, :], -240, 240).astype(NPF8)
    wgu8 = np.ascontiguousarray(np.concatenate([wg8, wu8], axis=1))
    if fp8_down:
        wd32 = np.asarray(expert_down_w, dtype=np.float32)
        sdw = F8MAX / np.maximum(np.abs(wd32).reshape(NEXP, -1).max(axis=1), 1e-9)
        wd_ship = np.ascontiguousarray(
            np.clip(wd32 * sdw[:, None, None], -240, 240).astype(NPF8))
    else:
        wd_ship = np.asarray(expert_down_w, dtype=np.float16)
        sdw = np.ones(NEXP, np.float32)
    convw16 = np.ascontiguousarray(
        np.transpose(np.asarray(conv_w, dtype=np.float16), (1, 2, 0))
    )  # (H, KS, I)
    swu16 = np.asarray(shared_up_w, dtype=np.float16)
    swd16 = np.asarray(shared_down_w, dtype=np.float16)
    flat16 = flat.astype(np.float16)
    x8_full = np.clip(flat * sx, -240, 240).astype(NPF8)  # (T, H)

    gsc = np.zeros((128, NEXP * MI), np.float32)
    usc = np.zeros((128, NEXP * MI), np.float32)
    for e in range(NEXP):
        for mi in range(MI):
            gsc[:, e * MI + mi] = 1.0 / (sx * sgw[e, mi * 128 : (mi + 1) * 128])
            usc[:, e * MI + mi] = 1.0 / (sx * suw[e, mi * 128 : (mi + 1) * 128])

    # identity scatter indices (rows 0..TC-1)
    ids_arr = _wrap_idxs(np.arange(TC, dtype=np.int64), TC)

    idcap = max([c for (e, c, _) in work if e == E - 1] or [128])
    in_maps2 = []
    for c in cores:
        lists = per_core_lists[c]
        idx_parts, g_parts, x8_parts = [], [], []
        xid_arr = np.zeros((128, idcap // 128, H), np.float16)
        for w, (e, cap, szk) in enumerate(work):
            k0 = chunk0[w]
            toks = lists[e][0][k0 : k0 + 512][:szk]
            gats = lists[e][1][k0 : k0 + 512][:szk]
            sz_c = len(toks)
            if sz_c < szk:
                # pad to the compiled scatter size; pads target the trash row
                # TC so they can never race with a real row's accumulation
                toks = np.concatenate([toks, np.full(szk - sz_c, TC, np.int64)])
                gats = np.concatenate([gats, np.zeros(szk - sz_c, np.float32)])
            gats_eff = gats / sdw[e] if (e != E - 1 and fp8_down) else gats
            idx_parts.append(_wrap_idxs(toks, cap))
            g_parts.append(_gate_cols(gats_eff, cap))
            xg = np.zeros((128, HJ, cap), NPF8)
            if len(toks):
                rows = x8_full[c * TC + np.minimum(toks, TC - 1)]  # (szk, H)
                xg[:, :, : len(toks)] = np.transpose(
                    rows.reshape(len(toks), HJ, 128), (2, 1, 0)
                )
            x8_parts.append(xg.reshape(128, HJ * cap))
            if e == E - 1 and len(toks):
                r16 = flat16[c * TC + np.minimum(toks, TC - 1)]
                n_ = len(toks)
                dst = np.zeros((-(-n_ // 128) * 128, H), np.float16)
                dst[:n_] = r16
                xid_arr[:, : -(-n_ // 128), :] = np.transpose(
                    dst.reshape(-1, 128, H), (1, 0, 2)
                )
        xTh = np.zeros((H, TC + 2), dtype=np.float16)
        xTh[:, 2:] = flat16[c * TC : (c + 1) * TC].T
        if (c * TC) % S != 0:
            xTh[:, 0:2] = flat16[c * TC - 2 : c * TC].T
        in_maps2.append(
            {
                "xTh": xTh,
                "convw": convw16,
                "swu": swu16,
                "swd": swd16,
                "wgu": wgu8,
                "wd": wd_ship,
                "x8": np.concatenate(x8_parts, axis=1),
                "xid": xid_arr,
                "tabi": np.concatenate(idx_parts + [ids_arr], axis=1),
                "tabf": np.concatenate(g_parts + [gsc, usc], axis=1),
            }
        )

    nc2 = _build_pass2(work, fp8_down=fp8_down)
    global NC2, IN_MAPS2
    NC2, IN_MAPS2 = nc2, in_maps2
    if dry:
        return None
    res2 = run_bass_kernel_spmd(nc2, in_maps2, cores).results

    out = np.concatenate([res2[c]["out"][:TC] for c in cores], axis=0)
    return out.reshape(B, S, H).astype(np.float32)
